# revision 1
# baseline (speedup 1.0000x reference)
"""Trainium2 Bass kernel for the relational GCN layer (gnn_message_passing).

Math (from the reference):
    out[n, e, i] = sum_k sum_m sum_d adj[n, m, k] * x[m, d, (i-k)%4] * W[d, e, k]

Factored for the PE (contraction dim must sit on SBUF partitions):
    X4[m, f]   = x.reshape(4096, 128)            with f = d*4 + j
    G_k[f, n]  = sum_m X4[m, f] * adj[n, m, k]   (the big 256 MB contraction)
    outT[c, n] = sum_k sum_f Wbig[f, k, c] * G_k[f, n]   with c = e*4 + i
    Wbig[d*4+j, k, e*4+i] = W[d, e, k] if j == (i-k)%4 else 0

Precision: fp32 matmuls on trn2 lower to 2 HW passes x 2 cycles/col (4x the
2-byte rate).  Instead adj and x are split into fp16 hi+lo pairs and G is
accumulated as 3 half-rate products (ah*xh + al*xh + ah*xl) in fp32 PSUM —
22 effective mantissa bits, measured ~5e-7 rel err (fp32-level), at 3
cycles/col of PE time.  The dropped al*xl term is ~2^-22 relative.

Sharding: 1D over the node (row) dim of adj/out — core c owns rows
[c*512, (c+1)*512).  x and the (tiny) weight are replicated.  adj is
pre-packed on the host into hi/lo fp16 tiles laid out exactly as the PE
streams them ([m-partition, n-free], contiguous per partition per DMA), so
the kernel runs at the HBM roofline with zero on-chip transposes.
"""

import numpy as np

N_CORES = 8
NODES = 4096
N_PER_CORE = NODES // N_CORES          # 512
F = 128                                # d*4+j
C = 128                                # e*4+i
MB = 32                                # m-chunks of 128 (4096 / 128)
R = 4
MB_COLS = R * 2 * N_PER_CORE           # (k, hl, nn) = 4096 fp16 per m-chunk
# DMA group sizes in m-chunks (1 m-chunk = 1 MB of hi+lo fp16).  PE
# consumption (~3.1 us/MB) matches DMA delivery, so fine granularity keeps
# the PE trailing the stream by at most one chunk.
GROUP_SIZES = [1] * 32
assert sum(GROUP_SIZES) == MB
XPIECES = 4                            # x4hl loaded in 4 pieces of 0.5 MB

_PATCHED = False
_PROG = None


def _patch_tile_drain():
    """This container's walrus build rejects >2 sync waits on one Drain;
    split the Tile end-of-context drain into one single-wait drain per proc
    (semantically identical: the SP engine observes each clock lane in
    sequence before the barrier)."""
    global _PATCHED
    if _PATCHED:
        return
    from concourse.tile import TileContext
    from concourse.vector_clock import ScopedClock, VectorClock
    from concourse.tile_scheduler import N_PROCS

    def _split_drain_and_barrier(self, tick_clock, wait_clock):
        g = tick_clock.global_clock
        for p in range(N_PROCS):
            if g[p] > 0:
                d = self.nc.sync.drain()
                pc = VectorClock([g[q] if q == p else 0 for q in range(N_PROCS)])
                wait_clock.add_sem_waits(d.ins, ScopedClock({None: pc}))
        self.nc.all_engine_barrier()
        assert self.sems is not None
        popped = self.nc._tile_sem_poison_stack.pop()
        assert popped is self._sem_poison
        self.nc.clear_and_free_semaphores(list(self.sems.allocated().values()))
        self.nc.all_engine_barrier()

    TileContext._drain_and_barrier = _split_drain_and_barrier
    _PATCHED = True


def _split_sync_waits(bir_bytes, max_waits=1):
    """This container's walrus build rejects instructions carrying more than
    ~2 sync waits.  Hoist all but one wait of any instruction onto standalone
    EventSemaphore instructions on the same engine immediately before it —
    the engine then observes the semaphores sequentially, which is
    semantically identical."""
    import json
    j = json.loads(bir_bytes)

    # normalize all debug records (top-level debug_table entries and inline
    # ant_debug dicts): their traceback/path strings vary by process context
    # and working directory, which would defeat the content-addressed NEFF
    # cache
    def scrub(o):
        if isinstance(o, dict):
            if "ant_traceback" in o or "filename" in o:
                for key, stub in (("filename", "kernel.py"),
                                  ("kernel_name", "k"), ("ant_traceback", "")):
                    if key in o:
                        o[key] = stub
                if "lineno" in o:
                    o["lineno"] = 0
            for v in o.values():
                scrub(v)
        elif isinstance(o, list):
            for v in o:
                scrub(v)

    scrub(j)
    n_new = 0
    for f in j.get("functions", []):
        for bb in f.get("blocks", []):
            out_insts = []
            for inst in bb.get("instructions", []):
                waits = (inst.get("sync_info") or {}).get("on_wait") or []
                if len(waits) > max_waits:
                    keep = waits[-max_waits:]
                    for w in waits[:-max_waits]:
                        n_new += 1
                        ev = {
                            "engine": inst["engine"],
                            "ins": [],
                            "name": f"{inst['name']}_wsplit{n_new}",
                            "opcode": "EventSemaphore",
                            "outs": [],
                            "sync_info": {"on_update": [], "on_wait": [w]},
                        }
                        if "debug" in inst:
                            ev["debug"] = inst["debug"]
                        out_insts.append(ev)
                    inst["sync_info"]["on_wait"] = keep
                out_insts.append(inst)
            bb["instructions"] = out_insts
    return json.dumps(j).encode()


def _install_neff_cache():
    """The bass_exec compile path bypasses libneuronxla's NEFF cache, so a
    fresh process pays the full ~3 min walrus compile every run.  Add a
    content-addressed cache keyed on the exact BIR bytes."""
    import hashlib, os, shutil
    import concourse.bass_utils as bu
    import concourse.bass2jax as b2j
    if getattr(bu, "_ant_bir_neff_cache", False):
        return
    orig = bu.compile_bir_kernel
    cache_dir = os.path.expanduser("~/.neuron-compile-cache/bass-bir-neff")
    os.makedirs(cache_dir, exist_ok=True)

    def cached(bir_json, tmpdir, neff_name="file.neff"):
        data = bir_json if isinstance(bir_json, bytes) else bir_json.encode()
        key = hashlib.sha256(data).hexdigest()
        cpath = os.path.join(cache_dir, key + ".neff")
        if os.path.exists(cpath):
            dst = os.path.join(tmpdir, neff_name)
            shutil.copy(cpath, dst)
            return dst
        neff = orig(bir_json, tmpdir, neff_name)
        try:
            shutil.copy(neff, cpath)
        except OSError:
            pass
        return neff

    bu.compile_bir_kernel = cached
    b2j.compile_bir_kernel = cached
    bu._ant_bir_neff_cache = True


def _build_program():
    global _PROG
    if _PROG is not None:
        return _PROG
    _patch_tile_drain()
    _install_neff_cache()
    import concourse.bass as bass
    import concourse.mybir as mybir
    from concourse.tile import TileContext

    f32 = mybir.dt.float32
    f16 = mybir.dt.float16
    nc = bass.Bass()
    # adjt[mb, mp, (hl, k, nn)]: hi/lo fp16 of adj[n0+nn, mb*128+mp, k]
    adjt = nc.dram_tensor("adjt", [MB, 128, MB_COLS], f16,
                          kind="ExternalInput")
    # x4hl[mp, xp, hl, mb8, f]: hi/lo fp16 of x.reshape(4096, 128)[mb*128+mp, f]
    # with mb = xp*(MB//XPIECES) + mb8; piece-major so each piece is one
    # contiguous-per-partition DMA
    x4hl = nc.dram_tensor("x4hl", [128, XPIECES, 2, MB // XPIECES, F], f16,
                          kind="ExternalInput")
    # wbigt[f, k, c] fp32
    wbigt = nc.dram_tensor("wbigt", [F, R, C], f32, kind="ExternalInput")
    outt = nc.dram_tensor("outt", [C, N_PER_CORE], f32, kind="ExternalOutput")

    with TileContext(nc) as tc:
        with (
            tc.tile_pool(name="const", bufs=1) as cpool,
            tc.tile_pool(name="adj", bufs=14) as apool,
            tc.tile_pool(name="gout", bufs=1) as gpool,
            tc.tile_pool(name="psum", bufs=1, space="PSUM") as ppool,
        ):
            # HAM warmup: dummy matmuls on a zeroed tile so the PE
            # clock-gate is at 8/8 by the time real data arrives (PE is
            # otherwise idle while the first DMA group lands).
            warm = cpool.tile([128, F], f16)
            nc.vector.memset(warm[:, :], 0.0)
            wps = ppool.tile([128, 64], f32, tag="warm")
            for _ in range(48):
                nc.tensor.matmul(wps[:, :], lhsT=warm[:, :], rhs=warm[:, :64],
                                 start=True, stop=True)

            # x piece 0 leads the SP ring (it gates the first matmul and the
            # ACT ring's sequencer starts ~3 us later); wbig + the remaining
            # x pieces ride the ACT ring
            xsb = cpool.tile([128, XPIECES, 2, MB // XPIECES, F], f16)
            nc.sync.dma_start(out=xsb[:, 0], in_=x4hl[:, 0])
            wsb = cpool.tile([F, R, C], f32)
            nc.scalar.dma_start(out=wsb[:, :, :], in_=wbigt[:, :, :])

            def xslice(mb, hl):
                return xsb[:, mb // (MB // XPIECES), hl, mb % (MB // XPIECES), :]

            gps = [ppool.tile([F, N_PER_CORE], f32, tag=f"g{k}", name=f"gps{k}")
                   for k in range(R)]

            def rhs(adjsb, a, k, hl):
                off = (hl * R + k) * N_PER_CORE
                return adjsb[:, a, off:off + N_PER_CORE]

            HALF = MB_COLS // 2
            mb0 = 0
            for g, gsz in enumerate(GROUP_SIZES):
                adjsb = apool.tile([128, max(GROUP_SIZES), MB_COLS], f16,
                                   tag="adjsb")
                # alternate the two HWDGE rings for throughput
                dma_eng = nc.sync if g % 2 == 0 else nc.scalar
                if g == 0:
                    # stream the first chunk as eight 128 KB (hl, k) pieces
                    # in exact matmul consumption order — the first matmul
                    # starts as soon as the first piece lands
                    for k in range(R):
                        for hl in range(2):
                            off = (hl * R + k) * N_PER_CORE
                            dma_eng.dma_start(
                                out=adjsb[:, 0, off:off + N_PER_CORE],
                                in_=adjt[0, :, off:off + N_PER_CORE])
                else:
                    dma_eng.dma_start(
                        out=adjsb[:, :gsz, :],
                        in_=adjt[mb0:mb0 + gsz].rearrange("a p c -> p a c"))
                if g in (3, 9, 15):
                    # remaining x pieces, due at mb 8/16/24, on the ACT ring
                    nc.scalar.dma_start(out=xsb[:, g // 6 + 1],
                                        in_=x4hl[:, g // 6 + 1])
                for a in range(gsz):
                    mb = mb0 + a
                    ks = range(R) if mb % 2 == 0 else range(R - 1, -1, -1)
                    if mb < MB - 1:
                        # (xh.ah, xh.al) pairs share stationary AND psum bank
                        # — no switch inside a pair; xl.ah sweep afterwards;
                        # serpentine k avoids a bank jump at chunk boundaries
                        for k in ks:
                            nc.tensor.matmul(gps[k][:, :], lhsT=xslice(mb, 0),
                                             rhs=rhs(adjsb, a, k, 0),
                                             start=(mb == 0), stop=False)
                            nc.tensor.matmul(gps[k][:, :], lhsT=xslice(mb, 0),
                                             rhs=rhs(adjsb, a, k, 1),
                                             start=False, stop=False)
                        for k in ks:
                            nc.tensor.matmul(gps[k][:, :], lhsT=xslice(mb, 1),
                                             rhs=rhs(adjsb, a, k, 0),
                                             start=False, stop=False)
                    else:
                        # last chunk: finish bank k completely before k+1 so
                        # each PSUM->SBUF copy (and the final matmuls) can
                        # start as early as possible
                        for k in ks:
                            nc.tensor.matmul(gps[k][:, :], lhsT=xslice(mb, 0),
                                             rhs=rhs(adjsb, a, k, 0),
                                             start=False, stop=False)
                            nc.tensor.matmul(gps[k][:, :], lhsT=xslice(mb, 0),
                                             rhs=rhs(adjsb, a, k, 1),
                                             start=False, stop=False)
                            nc.tensor.matmul(gps[k][:, :], lhsT=xslice(mb, 1),
                                             rhs=rhs(adjsb, a, k, 0),
                                             start=False, stop=True)
                mb0 += gsz

            gsb = gpool.tile([F, R, N_PER_CORE], f32)
            kcopy = range(R - 1, -1, -1) if (MB - 1) % 2 else range(R)
            for k in kcopy:
                nc.vector.tensor_copy(gsb[:, k, :], gps[k][:, :])

            # finals in two n-halves so the first half's copy + store overlap
            # the second half's matmuls
            NH = N_PER_CORE // 2
            osb = gpool.tile([C, N_PER_CORE], f32, tag="osb")
            for h in range(2):
                ops = ppool.tile([C, NH], f32, tag=f"out{h}", name=f"ops{h}")
                for k in range(R):
                    nc.tensor.matmul(ops[:, :], lhsT=wsb[:, k, :],
                                     rhs=gsb[:, k, h * NH:(h + 1) * NH],
                                     start=(k == 0), stop=(k == R - 1))
                nc.vector.tensor_copy(osb[:, h * NH:(h + 1) * NH], ops[:, :])
                nc.sync.dma_start(out=outt[:, h * NH:(h + 1) * NH],
                                  in_=osb[:, h * NH:(h + 1) * NH])

    _orig_to_json = nc.to_json_bytes
    nc.to_json_bytes = lambda: _split_sync_waits(_orig_to_json())

    _PROG = nc
    return nc


def _pack_adj(adj):
    """adj [4096, 4096, 4] f32 -> per-core [MB, 128, MB_COLS] fp16 hi/lo
    with adjt[c][mb, mp, (hl, k, nn)] = hl-part of adj[c*512+nn, mb*128+mp, k]."""
    A = adj.reshape(N_CORES, N_PER_CORE, MB, 128, R)
    At = np.ascontiguousarray(A.transpose(0, 2, 3, 4, 1))  # [c,mb,mp,k,nn]
    hi = At.astype(np.float16)
    lo = (At - hi.astype(np.float32)).astype(np.float16)
    out = np.empty((N_CORES, MB, 128, 2, R, N_PER_CORE), np.float16)
    out[:, :, :, 0] = hi
    out[:, :, :, 1] = lo
    return out.reshape(N_CORES, MB, 128, MB_COLS)


def _prepare_in_maps(x, adj, weight):
    x = np.ascontiguousarray(np.asarray(x), dtype=np.float32)
    adj = np.ascontiguousarray(np.asarray(adj), dtype=np.float32)
    weight = np.ascontiguousarray(np.asarray(weight), dtype=np.float32)

    x4 = np.ascontiguousarray(
        x.reshape(MB, 128, F).transpose(1, 0, 2))          # [mp, mb, f]
    x4r = x4.reshape(128, XPIECES, MB // XPIECES, F)
    x4hl = np.empty((128, XPIECES, 2, MB // XPIECES, F), np.float16)
    x4hl[:, :, 0] = x4r.astype(np.float16)
    x4hl[:, :, 1] = (x4r - x4hl[:, :, 0].astype(np.float32)).astype(np.float16)
    wbigt = np.zeros((F, R, C), np.float32)                # [f, k, c]
    for k in range(R):
        for i in range(R):
            j = (i - k) % R
            wbigt[j::R, k, i::R] = weight[:, :, k]
    adjt = _pack_adj(adj)
    return [{"adjt": adjt[c], "x4hl": x4hl, "wbigt": wbigt}
            for c in range(N_CORES)]


def _assemble_out(results):
    outt = np.stack([r["outt"] for r in results])          # [8, 128, 512]
    out = outt.reshape(N_CORES, 32, R, N_PER_CORE)         # [c, e, i, nn]
    out = out.transpose(0, 3, 1, 2).reshape(NODES, 32, R)  # [n, e, i]
    return np.ascontiguousarray(out)


def kernel(x, adj, weight):
    import os
    # the bass runner reaches the NeuronCores through the axon PJRT proxy;
    # make sure jax can initialize that platform (harmless if already set)
    plats = os.environ.get("JAX_PLATFORMS", "")
    if "axon" not in plats:
        os.environ["JAX_PLATFORMS"] = "axon,cpu" if not plats else f"axon,{plats}"
    nc = _build_program()
    in_maps = _prepare_in_maps(x, adj, weight)
    from concourse.bass_utils import run_bass_kernel_spmd
    res = run_bass_kernel_spmd(nc, in_maps, core_ids=list(range(N_CORES)))
    return _assemble_out(res.results)



# revision 2
# speedup vs baseline: 2.5159x; 2.5159x over previous
"""Trainium2 Bass kernel for the relational GCN layer (gnn_message_passing).

Math (from the reference):
    out[n, e, i] = sum_k sum_m sum_d adj[n, m, k] * x[m, d, (i-k)%4] * W[d, e, k]

Factored for the PE (contraction dim must sit on SBUF partitions):
    X4[m, f]   = x.reshape(4096, 128)            with f = d*4 + j
    G_k[f, n]  = sum_m X4[m, f] * adj[n, m, k]   (the big 256 MB contraction)
    outT[c, n] = sum_k sum_f Wbig[f, k, c] * G_k[f, n]   with c = e*4 + i
    Wbig[d*4+j, k, e*4+i] = W[d, e, k] if j == (i-k)%4 else 0

Precision: the rel-err budget is 2e-2, so adj is streamed as a SINGLE fp8
pass instead of fp16 hi/lo pairs.  adj ~ U[0,1) is centered and scaled
(16*(adj-0.5)) and quantized to float8_e3m4 — on uniform data e3m4 acts as
a ~6.2-bit uniform quantizer, and centering removes the mean so the exact
rank-1 term 0.5*sum_m x4[m,f] (host-computed in f64) is folded back in as
a per-partition bias on the output.  x rides as fp16 (near-exact), G and
Wbig/16 as bf16.  Measured end-to-end rel err ~7e-3 (threshold 2e-2).

This cuts HBM traffic 3.5x (32 MB -> 9.2 MB per core) and PE columns 3x
(one product per (chunk, k) instead of three) vs the hi/lo fp16 version.

Sharding: 1D over the node (row) dim of adj/out — core c owns rows
[c*512, (c+1)*512).  x, Wbig and the bias are replicated.  adj is packed
on the host into centered e3m4 tiles laid out exactly as the PE streams
them ([m-partition, (k, n)-free], contiguous per partition per DMA).
"""

import numpy as np
import ml_dtypes

N_CORES = 8
NODES = 4096
N_PER_CORE = NODES // N_CORES          # 512
F = 128                                # d*4+j
C = 128                                # e*4+i
MB = 32                                # m-chunks of 128 (4096 / 128)
R = 4
MB_COLS = R * N_PER_CORE               # (k, nn) = 2048 fp8 bytes per m-chunk
XPIECES = 4                            # x fp16 loaded in 4 pieces of 256 KB
ADJ_SCALE = 16.0                       # fp8 stores 16*(adj-0.5)

_PATCHED = False
_PROG = None


def _patch_tile_drain():
    """This container's walrus build rejects >2 sync waits on one Drain;
    split the Tile end-of-context drain into one single-wait drain per proc
    (semantically identical: the SP engine observes each clock lane in
    sequence before the barrier)."""
    global _PATCHED
    if _PATCHED:
        return
    from concourse.tile import TileContext
    from concourse.vector_clock import ScopedClock, VectorClock
    from concourse.tile_scheduler import N_PROCS

    def _split_drain_and_barrier(self, tick_clock, wait_clock):
        g = tick_clock.global_clock
        for p in range(N_PROCS):
            if g[p] > 0:
                d = self.nc.sync.drain()
                pc = VectorClock([g[q] if q == p else 0 for q in range(N_PROCS)])
                wait_clock.add_sem_waits(d.ins, ScopedClock({None: pc}))
        self.nc.all_engine_barrier()
        assert self.sems is not None
        popped = self.nc._tile_sem_poison_stack.pop()
        assert popped is self._sem_poison
        self.nc.clear_and_free_semaphores(list(self.sems.allocated().values()))
        self.nc.all_engine_barrier()

    TileContext._drain_and_barrier = _split_drain_and_barrier
    _PATCHED = True


def _split_sync_waits(bir_bytes, max_waits=1):
    """This container's walrus build rejects instructions carrying more than
    ~2 sync waits.  Hoist all but one wait of any instruction onto standalone
    EventSemaphore instructions on the same engine immediately before it —
    the engine then observes the semaphores sequentially, which is
    semantically identical."""
    import json
    j = json.loads(bir_bytes)

    # normalize all debug records (top-level debug_table entries and inline
    # ant_debug dicts): their traceback/path strings vary by process context
    # and working directory, which would defeat the content-addressed NEFF
    # cache
    def scrub(o):
        if isinstance(o, dict):
            if "ant_traceback" in o or "filename" in o:
                for key, stub in (("filename", "kernel.py"),
                                  ("kernel_name", "k"), ("ant_traceback", "")):
                    if key in o:
                        o[key] = stub
                if "lineno" in o:
                    o["lineno"] = 0
            for v in o.values():
                scrub(v)
        elif isinstance(o, list):
            for v in o:
                scrub(v)

    scrub(j)
    n_new = 0
    for f in j.get("functions", []):
        for bb in f.get("blocks", []):
            out_insts = []
            for inst in bb.get("instructions", []):
                waits = (inst.get("sync_info") or {}).get("on_wait") or []
                if len(waits) > max_waits:
                    keep = waits[-max_waits:]
                    for w in waits[:-max_waits]:
                        n_new += 1
                        ev = {
                            "engine": inst["engine"],
                            "ins": [],
                            "name": f"{inst['name']}_wsplit{n_new}",
                            "opcode": "EventSemaphore",
                            "outs": [],
                            "sync_info": {"on_update": [], "on_wait": [w]},
                        }
                        if "debug" in inst:
                            ev["debug"] = inst["debug"]
                        out_insts.append(ev)
                    inst["sync_info"]["on_wait"] = keep
                out_insts.append(inst)
            bb["instructions"] = out_insts
    return json.dumps(j).encode()


def _install_neff_cache():
    """The bass_exec compile path bypasses libneuronxla's NEFF cache, so a
    fresh process pays the full ~3 min walrus compile every run.  Add a
    content-addressed cache keyed on the exact BIR bytes."""
    import hashlib, os, shutil
    import concourse.bass_utils as bu
    import concourse.bass2jax as b2j
    if getattr(bu, "_ant_bir_neff_cache", False):
        return
    orig = bu.compile_bir_kernel
    cache_dir = os.path.expanduser("~/.neuron-compile-cache/bass-bir-neff")
    os.makedirs(cache_dir, exist_ok=True)

    def cached(bir_json, tmpdir, neff_name="file.neff"):
        data = bir_json if isinstance(bir_json, bytes) else bir_json.encode()
        key = hashlib.sha256(data).hexdigest()
        cpath = os.path.join(cache_dir, key + ".neff")
        if os.path.exists(cpath):
            dst = os.path.join(tmpdir, neff_name)
            shutil.copy(cpath, dst)
            return dst
        neff = orig(bir_json, tmpdir, neff_name)
        try:
            shutil.copy(neff, cpath)
        except OSError:
            pass
        return neff

    bu.compile_bir_kernel = cached
    b2j.compile_bir_kernel = cached
    bu._ant_bir_neff_cache = True


def _build_program():
    global _PROG
    if _PROG is not None:
        return _PROG
    _patch_tile_drain()
    _install_neff_cache()
    import concourse.bass as bass
    import concourse.mybir as mybir
    from concourse.tile import TileContext

    f32 = mybir.dt.float32
    f16 = mybir.dt.float16
    bf16 = mybir.dt.bfloat16
    f8 = mybir.dt.float8e3
    nc = bass.Bass()
    # adjt[mb, mp, (k, nn)]: e3m4 of 16*(adj[n0+nn, mb*128+mp, k] - 0.5)
    adjt = nc.dram_tensor("adjt", [MB, 128, MB_COLS], f8, kind="ExternalInput")
    # xt[mp, xp, mb8, f]: fp16 of x.reshape(4096, 128)[mb*128+mp, f]
    # with mb = xp*(MB//XPIECES) + mb8; piece-major so each piece is one
    # contiguous-per-partition DMA
    xt = nc.dram_tensor("xt", [128, XPIECES, MB // XPIECES, F], f16,
                        kind="ExternalInput")
    # x0t: duplicate of the mb=0 slice, tiny first DMA so matmul 0 starts
    # without waiting for the full 256 KB piece 0
    x0t = nc.dram_tensor("x0t", [128, F], f16, kind="ExternalInput")
    # wt[f, k, c] = Wbig/ADJ_SCALE in bf16
    wt = nc.dram_tensor("wt", [F, R, C], bf16, kind="ExternalInput")
    # bt[c]: exact rank-1 bias sum_k,f (0.5*sum_m x4[m,f]) * Wbig[f,k,c]
    bt = nc.dram_tensor("bt", [C, 1], f32, kind="ExternalInput")
    outt = nc.dram_tensor("outt", [C, N_PER_CORE], f32, kind="ExternalOutput")

    with TileContext(nc) as tc:
        with (
            tc.tile_pool(name="const", bufs=1) as cpool,
            tc.tile_pool(name="adj", bufs=16) as apool,
            tc.tile_pool(name="gout", bufs=1) as gpool,
            tc.tile_pool(name="psum", bufs=1, space="PSUM") as ppool,
        ):
            # HAM warmup: dummy matmuls on a zeroed tile so the PE
            # clock-gate is at 8/8 by the time real data arrives (PE is
            # otherwise idle while the first DMA group lands).
            warm = cpool.tile([128, F], f16)
            nc.vector.memset(warm[:, :], 0.0)
            wps = ppool.tile([128, 64], f32, tag="warm")
            for _ in range(48):
                nc.tensor.matmul(wps[:, :], lhsT=warm[:, :], rhs=warm[:, :64],
                                 start=True, stop=True)

            # mb0 x slice leads the SP ring (it gates the first matmul);
            # Wbig + bias ride the gpsimd (SWDGE) ring — they are only
            # needed in the tail and this keeps the HWDGE rings for adj
            x0sb = cpool.tile([128, F], f16)
            nc.sync.dma_start(out=x0sb[:, :], in_=x0t[:, :])
            xsb = cpool.tile([128, XPIECES, MB // XPIECES, F], f16)
            wsb = cpool.tile([F, R, C], bf16)
            nc.gpsimd.dma_start(out=wsb[:, :, :], in_=wt[:, :, :])
            bsb = cpool.tile([C, 1], f32)
            nc.gpsimd.dma_start(out=bsb[:, :], in_=bt[:, :])

            gps = [ppool.tile([F, N_PER_CORE], f32, tag=f"g{k}", name=f"gps{k}")
                   for k in range(R)]

            for g in range(MB):
                adjsb = apool.tile([128, MB_COLS], f8, tag="adjsb")
                # alternate the two HWDGE rings for throughput
                dma_eng = nc.sync if g % 2 == 0 else nc.scalar
                if g == 0:
                    # stream the first chunk as four 64 KB k-pieces in
                    # exact matmul consumption order — the first matmul
                    # starts as soon as the first piece lands
                    for k in range(R):
                        off = k * N_PER_CORE
                        dma_eng.dma_start(
                            out=adjsb[:, off:off + N_PER_CORE],
                            in_=adjt[0, :, off:off + N_PER_CORE])
                    # x piece 0 queues behind chunk 0 on the SP ring
                    nc.sync.dma_start(out=xsb[:, 0], in_=xt[:, 0])
                else:
                    dma_eng.dma_start(out=adjsb[:, :], in_=adjt[g])
                if g in (3, 9, 15):
                    # remaining x pieces, due at mb 8/16/24, on the ACT ring
                    nc.scalar.dma_start(out=xsb[:, g // 6 + 1],
                                        in_=xt[:, g // 6 + 1])
                lhsT = x0sb[:, :] if g == 0 else \
                    xsb[:, g // (MB // XPIECES), g % (MB // XPIECES), :]
                # serpentine k avoids a psum bank jump at chunk boundaries
                ks = range(R) if g % 2 == 0 else range(R - 1, -1, -1)
                for k in ks:
                    nc.tensor.matmul(gps[k][:, :], lhsT=lhsT,
                                     rhs=adjsb[:, k * N_PER_CORE:
                                               (k + 1) * N_PER_CORE],
                                     start=(g == 0), stop=(g == MB - 1))

            # PSUM -> SBUF as bf16 for the cheap second stage, in the order
            # the last chunk's matmuls complete
            gsb = gpool.tile([F, R, N_PER_CORE], bf16)
            kcopy = range(R - 1, -1, -1) if (MB - 1) % 2 else range(R)
            for k in kcopy:
                nc.vector.tensor_copy(gsb[:, k, :], gps[k][:, :])

            # finals in two n-halves so the first half's copy + store overlap
            # the second half's matmuls; bias folds into the PSUM->SBUF copy
            NH = N_PER_CORE // 2
            osb = gpool.tile([C, N_PER_CORE], f32, tag="osb")
            for h in range(2):
                ops = ppool.tile([C, NH], f32, tag=f"out{h}", name=f"ops{h}")
                for k in range(R):
                    nc.tensor.matmul(ops[:, :], lhsT=wsb[:, k, :],
                                     rhs=gsb[:, k, h * NH:(h + 1) * NH],
                                     start=(k == 0), stop=(k == R - 1))
                nc.vector.tensor_scalar_add(osb[:, h * NH:(h + 1) * NH],
                                            ops[:, :], bsb[:, :])
                nc.sync.dma_start(out=outt[:, h * NH:(h + 1) * NH],
                                  in_=osb[:, h * NH:(h + 1) * NH])

    _orig_to_json = nc.to_json_bytes
    nc.to_json_bytes = lambda: _split_sync_waits(_orig_to_json())

    _PROG = nc
    return nc


def _pack_adj(adj):
    """adj [4096, 4096, 4] f32 -> per-core [MB, 128, MB_COLS] e3m4 with
    adjt[c][mb, mp, (k, nn)] = e3m4(16*(adj[c*512+nn, mb*128+mp, k] - 0.5))."""
    A = adj.reshape(N_CORES, N_PER_CORE, MB, 128, R)
    At = np.ascontiguousarray(A.transpose(0, 2, 3, 4, 1))  # [c,mb,mp,k,nn]
    q = ((At - np.float32(0.5)) * np.float32(ADJ_SCALE)).astype(
        ml_dtypes.float8_e3m4)
    return q.reshape(N_CORES, MB, 128, MB_COLS)


def _prepare_in_maps(x, adj, weight):
    x = np.ascontiguousarray(np.asarray(x), dtype=np.float32)
    adj = np.ascontiguousarray(np.asarray(adj), dtype=np.float32)
    weight = np.asarray(weight).astype(np.float64)

    x4 = x.reshape(NODES, F)                               # [m, f], f = d*4+j
    x4t = np.ascontiguousarray(
        x4.reshape(MB, 128, F).transpose(1, 0, 2))         # [mp, mb, f]
    xt = x4t.reshape(128, XPIECES, MB // XPIECES, F).astype(np.float16)
    x0t = np.ascontiguousarray(x4t[:, 0, :]).astype(np.float16)

    wbig = np.zeros((F, R, C), np.float64)                 # [f, k, c]
    for k in range(R):
        for i in range(R):
            j = (i - k) % R
            wbig[j::R, k, i::R] = weight[:, :, k]
    wt = (wbig / ADJ_SCALE).astype(ml_dtypes.bfloat16)

    bias_f = 0.5 * x4.astype(np.float64).sum(axis=0)       # [f]
    b_out = np.einsum('f,fkc->c', bias_f, wbig)            # [c]
    bt = np.ascontiguousarray(b_out.astype(np.float32).reshape(C, 1))

    adjq = _pack_adj(adj)
    return [{"adjt": adjq[c], "xt": xt, "x0t": x0t, "wt": wt, "bt": bt}
            for c in range(N_CORES)]


def _assemble_out(results):
    outt = np.stack([r["outt"] for r in results])          # [8, 128, 512]
    out = outt.reshape(N_CORES, 32, R, N_PER_CORE)         # [c, e, i, nn]
    out = out.transpose(0, 3, 1, 2).reshape(NODES, 32, R)  # [n, e, i]
    return np.ascontiguousarray(out)


def kernel(x, adj, weight):
    import os
    # the bass runner reaches the NeuronCores through the axon PJRT proxy;
    # make sure jax can initialize that platform (harmless if already set)
    plats = os.environ.get("JAX_PLATFORMS", "")
    if "axon" not in plats:
        os.environ["JAX_PLATFORMS"] = "axon,cpu" if not plats else f"axon,{plats}"
    nc = _build_program()
    in_maps = _prepare_in_maps(x, adj, weight)
    from concourse.bass_utils import run_bass_kernel_spmd
    res = run_bass_kernel_spmd(nc, in_maps, core_ids=list(range(N_CORES)))
    return _assemble_out(res.results)


# revision 7
# speedup vs baseline: 2.5213x; 1.0021x over previous
"""Trainium2 Bass kernel for the relational GCN layer (gnn_message_passing).

Math (from the reference):
    out[n, e, i] = sum_k sum_m sum_d adj[n, m, k] * x[m, d, (i-k)%4] * W[d, e, k]

Factored for the PE (contraction dim must sit on SBUF partitions):
    X4[m, f]   = x.reshape(4096, 128)            with f = d*4 + j
    G_k[f, n]  = sum_m X4[m, f] * adj[n, m, k]   (the big 256 MB contraction)
    outT[c, n] = sum_k sum_f Wbig[f, k, c] * G_k[f, n]   with c = e*4 + i
    Wbig[d*4+j, k, e*4+i] = W[d, e, k] if j == (i-k)%4 else 0

Precision: the rel-err budget is 2e-2, so adj is streamed as a SINGLE fp8
pass instead of fp16 hi/lo pairs.  adj ~ U[0,1) is centered and scaled
(16*(adj-0.5)) and quantized to float8_e3m4 — on uniform data e3m4 acts as
a ~6.2-bit uniform quantizer, and centering removes the mean so the exact
rank-1 term 0.5*sum_m x4[m,f] (host-computed in f64) is folded back in as
a per-partition bias on the output.  x rides as fp16 (near-exact), G and
Wbig/16 as bf16.  Measured end-to-end rel err ~7e-3 (threshold 2e-2).

This cuts HBM traffic 3.5x (32 MB -> 9.2 MB per core) and PE columns 3x
(one product per (chunk, k) instead of three) vs the hi/lo fp16 version.

Sharding: 1D over the node (row) dim of adj/out — core c owns rows
[c*512, (c+1)*512).  x, Wbig and the bias are replicated.  adj is packed
on the host into centered e3m4 tiles laid out exactly as the PE streams
them ([m-partition, (k, n)-free], contiguous per partition per DMA).
"""

import numpy as np
import ml_dtypes

N_CORES = 8
NODES = 4096
N_PER_CORE = NODES // N_CORES          # 512
F = 128                                # d*4+j
C = 128                                # e*4+i
MB = 32                                # m-chunks of 128 (4096 / 128)
R = 4
MB_COLS = R * N_PER_CORE               # (k, nn) = 2048 fp8 bytes per m-chunk
XPIECES = 4                            # x fp16 loaded in 4 pieces of 256 KB
ADJ_SCALE = 16.0                       # fp8 stores 16*(adj-0.5)

_PATCHED = False
_PROG = None


def _patch_tile_drain():
    """This container's walrus build rejects >2 sync waits on one Drain;
    split the Tile end-of-context drain into one single-wait drain per proc
    (semantically identical: the SP engine observes each clock lane in
    sequence before the barrier)."""
    global _PATCHED
    if _PATCHED:
        return
    from concourse.tile import TileContext
    from concourse.vector_clock import ScopedClock, VectorClock
    from concourse.tile_scheduler import N_PROCS

    def _split_drain_and_barrier(self, tick_clock, wait_clock):
        g = tick_clock.global_clock
        for p in range(N_PROCS):
            if g[p] > 0:
                d = self.nc.sync.drain()
                pc = VectorClock([g[q] if q == p else 0 for q in range(N_PROCS)])
                wait_clock.add_sem_waits(d.ins, ScopedClock({None: pc}))
        self.nc.all_engine_barrier()
        assert self.sems is not None
        popped = self.nc._tile_sem_poison_stack.pop()
        assert popped is self._sem_poison
        self.nc.clear_and_free_semaphores(list(self.sems.allocated().values()))
        self.nc.all_engine_barrier()

    TileContext._drain_and_barrier = _split_drain_and_barrier
    _PATCHED = True


def _split_sync_waits(bir_bytes, max_waits=1):
    """This container's walrus build rejects instructions carrying more than
    ~2 sync waits.  Hoist all but one wait of any instruction onto standalone
    EventSemaphore instructions on the same engine immediately before it —
    the engine then observes the semaphores sequentially, which is
    semantically identical."""
    import json
    j = json.loads(bir_bytes)

    # normalize all debug records (top-level debug_table entries and inline
    # ant_debug dicts): their traceback/path strings vary by process context
    # and working directory, which would defeat the content-addressed NEFF
    # cache
    def scrub(o):
        if isinstance(o, dict):
            if "ant_traceback" in o or "filename" in o:
                for key, stub in (("filename", "kernel.py"),
                                  ("kernel_name", "k"), ("ant_traceback", "")):
                    if key in o:
                        o[key] = stub
                if "lineno" in o:
                    o["lineno"] = 0
            for v in o.values():
                scrub(v)
        elif isinstance(o, list):
            for v in o:
                scrub(v)

    scrub(j)
    n_new = 0
    for f in j.get("functions", []):
        for bb in f.get("blocks", []):
            out_insts = []
            for inst in bb.get("instructions", []):
                waits = (inst.get("sync_info") or {}).get("on_wait") or []
                if len(waits) > max_waits:
                    keep = waits[-max_waits:]
                    for w in waits[:-max_waits]:
                        n_new += 1
                        ev = {
                            "engine": inst["engine"],
                            "ins": [],
                            "name": f"{inst['name']}_wsplit{n_new}",
                            "opcode": "EventSemaphore",
                            "outs": [],
                            "sync_info": {"on_update": [], "on_wait": [w]},
                        }
                        if "debug" in inst:
                            ev["debug"] = inst["debug"]
                        out_insts.append(ev)
                    inst["sync_info"]["on_wait"] = keep
                out_insts.append(inst)
            bb["instructions"] = out_insts
    return json.dumps(j).encode()


def _install_neff_cache():
    """The bass_exec compile path bypasses libneuronxla's NEFF cache, so a
    fresh process pays the full ~3 min walrus compile every run.  Add a
    content-addressed cache keyed on the exact BIR bytes."""
    import hashlib, os, shutil
    import concourse.bass_utils as bu
    import concourse.bass2jax as b2j
    if getattr(bu, "_ant_bir_neff_cache", False):
        return
    orig = bu.compile_bir_kernel
    cache_dir = os.path.expanduser("~/.neuron-compile-cache/bass-bir-neff")
    os.makedirs(cache_dir, exist_ok=True)

    def cached(bir_json, tmpdir, neff_name="file.neff"):
        data = bir_json if isinstance(bir_json, bytes) else bir_json.encode()
        key = hashlib.sha256(data).hexdigest()
        cpath = os.path.join(cache_dir, key + ".neff")
        if os.path.exists(cpath):
            dst = os.path.join(tmpdir, neff_name)
            shutil.copy(cpath, dst)
            return dst
        neff = orig(bir_json, tmpdir, neff_name)
        try:
            shutil.copy(neff, cpath)
        except OSError:
            pass
        return neff

    bu.compile_bir_kernel = cached
    b2j.compile_bir_kernel = cached
    bu._ant_bir_neff_cache = True


def _build_program():
    global _PROG
    if _PROG is not None:
        return _PROG
    _patch_tile_drain()
    _install_neff_cache()
    import concourse.bass as bass
    import concourse.mybir as mybir
    from concourse.tile import TileContext

    f32 = mybir.dt.float32
    f16 = mybir.dt.float16
    bf16 = mybir.dt.bfloat16
    f8 = mybir.dt.float8e3
    nc = bass.Bass()
    # adjt[mb, mp, (k, nn)]: e3m4 of 16*(adj[n0+nn, mb*128+mp, k] - 0.5)
    adjt = nc.dram_tensor("adjt", [MB, 128, MB_COLS], f8, kind="ExternalInput")
    # xt[mp, xp, mb8, f]: fp16 of x.reshape(4096, 128)[mb*128+mp, f]
    # with mb = xp*(MB//XPIECES) + mb8; piece-major so each piece is one
    # contiguous-per-partition DMA
    xt = nc.dram_tensor("xt", [128, XPIECES, MB // XPIECES, F], f16,
                        kind="ExternalInput")
    # wt[f, k, c] = Wbig/ADJ_SCALE in bf16
    wt = nc.dram_tensor("wt", [F, R, C], bf16, kind="ExternalInput")
    # bt[c]: exact rank-1 bias sum_k,f (0.5*sum_m x4[m,f]) * Wbig[f,k,c]
    bt = nc.dram_tensor("bt", [C, 1], f32, kind="ExternalInput")
    outt = nc.dram_tensor("outt", [C, N_PER_CORE], f32, kind="ExternalOutput")

    with TileContext(nc) as tc:
        with (
            tc.tile_pool(name="const", bufs=1) as cpool,
            tc.tile_pool(name="adj", bufs=16) as apool,
            tc.tile_pool(name="gout", bufs=1) as gpool,
            tc.tile_pool(name="psum", bufs=1, space="PSUM") as ppool,
        ):
            # HAM warmup: a single accumulation chain of dummy matmuls so
            # the PE is busy (and the clock-gate accumulating credit) while
            # the first DMAs land; sized to end ~when chunk 0 arrives.
            warm = cpool.tile([128, F], f16)
            nc.vector.memset(warm[:, :], 0.0)
            wps = ppool.tile([128, 64], f32, tag="warm")
            NWARM = 40
            for i in range(NWARM):
                nc.tensor.matmul(wps[:, :], lhsT=warm[:, :], rhs=warm[:, :64],
                                 start=(i == 0), stop=(i == NWARM - 1))

            # x piece 0 leads the SP ring (it gates the first matmul);
            # Wbig + bias ride the gpsimd (SWDGE) ring — they are only
            # needed in the tail and this keeps the HWDGE rings for adj
            xsb = cpool.tile([128, XPIECES, MB // XPIECES, F], f16)
            nc.sync.dma_start(out=xsb[:, 0], in_=xt[:, 0])
            wsb = cpool.tile([F, R, C], bf16)
            nc.gpsimd.dma_start(out=wsb[:, :, :], in_=wt[:, :, :])
            bsb = cpool.tile([C, 1], f32)
            nc.gpsimd.dma_start(out=bsb[:, :], in_=bt[:, :])

            gps = [ppool.tile([F, N_PER_CORE], f32, tag=f"g{k}", name=f"gps{k}")
                   for k in range(R)]

            for g in range(MB):
                adjsb = apool.tile([128, MB_COLS], f8, tag="adjsb")
                # alternate the two HWDGE rings for throughput
                dma_eng = nc.sync if g % 2 == 0 else nc.scalar
                dma_eng.dma_start(out=adjsb[:, :], in_=adjt[g])
                if g in (3, 9, 15):
                    # remaining x pieces, due at mb 8/16/24, on the ACT ring
                    nc.scalar.dma_start(out=xsb[:, g // 6 + 1],
                                        in_=xt[:, g // 6 + 1])
                lhsT = xsb[:, g // (MB // XPIECES), g % (MB // XPIECES), :]
                # serpentine k avoids a psum bank jump at chunk boundaries
                ks = range(R) if g % 2 == 0 else range(R - 1, -1, -1)
                for k in ks:
                    nc.tensor.matmul(gps[k][:, :], lhsT=lhsT,
                                     rhs=adjsb[:, k * N_PER_CORE:
                                               (k + 1) * N_PER_CORE],
                                     start=(g == 0), stop=(g == MB - 1))

            # Tail: PSUM G -> SBUF bf16 in 8 (k, half) pieces split across
            # DVE and ACT so the casts run in parallel, each its own tile so
            # the stage-2 matmuls chase individual casts (not the full set);
            # k ordered as the last chunk's matmuls complete.
            NH = N_PER_CORE // 2
            kcopy = list(range(R - 1, -1, -1)) if (MB - 1) % 2 else list(range(R))
            import concourse.mybir as mybir_
            Copy = mybir_.ActivationFunctionType.Copy
            gkh = {}
            for k in kcopy:
                for h in range(2):
                    gkh[(k, h)] = gpool.tile([F, NH], bf16, tag=f"g{k}{h}",
                                             name=f"gkh{k}{h}")
                nc.vector.tensor_copy(gkh[(k, 0)][:, :], gps[k][:, :NH])
                nc.scalar.activation(gkh[(k, 1)][:, :], gps[k][:, NH:], Copy)

            # finals: both halves' matmuls chase the casts k-by-k; bias is
            # folded into the PSUM->SBUF copy; the two halves ship on
            # different HWDGE rings
            osb = gpool.tile([C, N_PER_CORE], f32, tag="osb")
            ops = [ppool.tile([C, NH], f32, tag=f"out{h}", name=f"ops{h}")
                   for h in range(2)]
            for ki, k in enumerate(kcopy):
                for h in range(2):
                    nc.tensor.matmul(ops[h][:, :], lhsT=wsb[:, k, :],
                                     rhs=gkh[(k, h)][:, :],
                                     start=(ki == 0), stop=(ki == R - 1))
            for h, eng in ((0, nc.sync), (1, nc.scalar)):
                nc.vector.tensor_scalar_add(osb[:, h * NH:(h + 1) * NH],
                                            ops[h][:, :], bsb[:, :])
                eng.dma_start(out=outt[:, h * NH:(h + 1) * NH],
                              in_=osb[:, h * NH:(h + 1) * NH])

    _orig_to_json = nc.to_json_bytes
    nc.to_json_bytes = lambda: _split_sync_waits(_orig_to_json())

    _PROG = nc
    return nc


def _pack_adj(adj):
    """adj [4096, 4096, 4] f32 -> per-core [MB, 128, MB_COLS] e3m4 with
    adjt[c][mb, mp, (k, nn)] = e3m4(16*(adj[c*512+nn, mb*128+mp, k] - 0.5))."""
    A = adj.reshape(N_CORES, N_PER_CORE, MB, 128, R)
    At = np.ascontiguousarray(A.transpose(0, 2, 3, 4, 1))  # [c,mb,mp,k,nn]
    q = ((At - np.float32(0.5)) * np.float32(ADJ_SCALE)).astype(
        ml_dtypes.float8_e3m4)
    return q.reshape(N_CORES, MB, 128, MB_COLS)


def _prepare_in_maps(x, adj, weight):
    x = np.ascontiguousarray(np.asarray(x), dtype=np.float32)
    adj = np.ascontiguousarray(np.asarray(adj), dtype=np.float32)
    weight = np.asarray(weight).astype(np.float64)

    x4 = x.reshape(NODES, F)                               # [m, f], f = d*4+j
    x4t = np.ascontiguousarray(
        x4.reshape(MB, 128, F).transpose(1, 0, 2))         # [mp, mb, f]
    xt = x4t.reshape(128, XPIECES, MB // XPIECES, F).astype(np.float16)

    wbig = np.zeros((F, R, C), np.float64)                 # [f, k, c]
    for k in range(R):
        for i in range(R):
            j = (i - k) % R
            wbig[j::R, k, i::R] = weight[:, :, k]
    wt = (wbig / ADJ_SCALE).astype(ml_dtypes.bfloat16)

    bias_f = 0.5 * x4.astype(np.float64).sum(axis=0)       # [f]
    b_out = np.einsum('f,fkc->c', bias_f, wbig)            # [c]
    bt = np.ascontiguousarray(b_out.astype(np.float32).reshape(C, 1))

    adjq = _pack_adj(adj)
    return [{"adjt": adjq[c], "xt": xt, "wt": wt, "bt": bt}
            for c in range(N_CORES)]


def _assemble_out(results):
    outt = np.stack([r["outt"] for r in results])          # [8, 128, 512]
    out = outt.reshape(N_CORES, 32, R, N_PER_CORE)         # [c, e, i, nn]
    out = out.transpose(0, 3, 1, 2).reshape(NODES, 32, R)  # [n, e, i]
    return np.ascontiguousarray(out)


def kernel(x, adj, weight):
    import os
    # the bass runner reaches the NeuronCores through the axon PJRT proxy;
    # make sure jax can initialize that platform (harmless if already set)
    plats = os.environ.get("JAX_PLATFORMS", "")
    if "axon" not in plats:
        os.environ["JAX_PLATFORMS"] = "axon,cpu" if not plats else f"axon,{plats}"
    nc = _build_program()
    in_maps = _prepare_in_maps(x, adj, weight)
    from concourse.bass_utils import run_bass_kernel_spmd
    res = run_bass_kernel_spmd(nc, in_maps, core_ids=list(range(N_CORES)))
    return _assemble_out(res.results)
